# revision 2
# baseline (speedup 1.0000x reference)
"""GCN layer (4-relation message passing) on 8 Trainium2 NeuronCores.

out = sum_r (A_r @ inp) @ W_r + sum_r b_r,  A_r in COO form (dst, src, val).

Sharding: edges sharded by dst range; core c owns dst in [c*12500, (c+1)*12500).

v2 design (vs v1 which shipped a [128,64] bf16 one-hot selection slab per
128-edge block): the selection matrix is built ON DEVICE, so per edge the
host ships only the 64-col bf16 message row (pure placement of inp[src])
plus 2+2 bytes of metadata (wloc, val as bf16).

Per (32-node dst window w, relation r) cell, per 128-edge block b:
  sel_b[p, j] = val_p * (j == wloc_p)   built by two wide DVE passes:
                is_equal(iota, wloc_bcast) then mult by val_bcast
  PE: aggT_wr [64feat, 32nodes] += MSG_b^T @ sel_b  (PSUM, col-tiled:
      rel 0/1 -> PSUM partitions 0:64 / 64:128 of one [128,32] tile)
Stage 2 per window: po [32nodes, 64outf] = aggT_01.T-stack @ [W0;W1]
  + aggT_23-stack @ [W2;W3] + ones.T @ bias   (bias summed on device).
Host: unscramble windows, no arithmetic beyond dtype casts.
"""

import math
from contextlib import ExitStack

import numpy as np

import concourse.bass as bass
import concourse.tile as tile
from concourse import bacc, mybir
from concourse.bass_utils import run_bass_kernel_spmd

# problem constants
N_NODES = 100000
N_REL = 4
N_EDGES = 1600000
IN_SIZE = 64
OUT_SIZE = 64

N_CORES = 8
NPC = N_NODES // N_CORES  # nodes (dst) per core
P = 128                   # partitions / edges per block
W = 32                    # dst-window width (nodes per psum column range)
GW = 40                   # blocks per DMA group

F32 = mybir.dt.float32
BF16 = mybir.dt.bfloat16


def _bf16():
    import ml_dtypes
    return ml_dtypes.bfloat16


def _host_prep(inp, src, dst, edge_val):
    """Bucket/pad edges per (core, window, rel); build msg/wloc/val slabs."""
    n_win = math.ceil(NPC / W)
    ncell = n_win * N_REL
    srcf = src.reshape(-1).astype(np.int64)
    dstf = dst.reshape(-1).astype(np.int64)
    valf = edge_val.reshape(-1).astype(np.float32)
    rel = np.repeat(np.arange(N_REL, dtype=np.int64), src.shape[1])

    core = dstf // NPC
    dloc = dstf % NPC
    win = dloc // W
    wloc = dloc % W
    cell = win * N_REL + rel
    key = core * ncell + cell

    counts = np.bincount(key, minlength=N_CORES * ncell).reshape(
        N_CORES, ncell)
    B = np.maximum((counts.max(axis=0) + P - 1) // P, 1).astype(np.int64)
    starts = np.zeros(ncell + 1, dtype=np.int64)
    np.cumsum(B, out=starts[1:])
    T = int(starts[-1])

    bf16 = _bf16()
    msg_all = np.zeros((N_CORES, P, T, IN_SIZE), dtype=bf16)
    wloc_all = np.full((N_CORES, P, T), -1.0, dtype=bf16)
    val_all = np.zeros((N_CORES, P, T), dtype=bf16)

    order = np.argsort(key, kind="stable")
    grp_start = np.zeros(N_CORES * ncell, dtype=np.int64)
    np.cumsum(counts.reshape(-1)[:-1], out=grp_start[1:])
    j = np.arange(len(order), dtype=np.int64) - grp_start[key[order]]
    t_col = starts[cell[order]] + (j // P)
    p_row = j % P
    c_ord = core[order]
    msg_all[c_ord, p_row, t_col, :] = inp[srcf[order]].astype(bf16)
    wloc_all[c_ord, p_row, t_col] = wloc[order].astype(bf16)
    val_all[c_ord, p_row, t_col] = valf[order].astype(bf16)

    return n_win, starts, T, msg_all, wloc_all, val_all


_PROG_CACHE = {}


def _build_program(n_win, starts, T):
    key = (W, GW, n_win, tuple(int(s) for s in starts), T)
    if key in _PROG_CACHE:
        return _PROG_CACHE[key]

    nc = bacc.Bacc("TRN2", target_bir_lowering=False, debug=False,
                   num_devices=N_CORES)
    wst = nc.dram_tensor("wst", [2 * IN_SIZE, 2 * OUT_SIZE], F32,
                         kind="ExternalInput").ap()
    biasd = nc.dram_tensor("biasd", [N_REL, OUT_SIZE], F32,
                           kind="ExternalInput").ap()
    onesd = nc.dram_tensor("onesd", [N_REL, W], F32,
                           kind="ExternalInput").ap()
    iotad = nc.dram_tensor("iotad", [P, GW * W], BF16,
                           kind="ExternalInput").ap()
    emsg = nc.dram_tensor("emsg", [P, T * IN_SIZE], BF16,
                          kind="ExternalInput").ap()
    ewlc = nc.dram_tensor("ewlc", [P, T], BF16, kind="ExternalInput").ap()
    eval_ = nc.dram_tensor("eval", [P, T], BF16, kind="ExternalInput").ap()
    n_wcol = (n_win + 3) // 4
    out = nc.dram_tensor("out", [P, n_wcol * OUT_SIZE], F32,
                         kind="ExternalOutput").ap()

    ngroups = (T + GW - 1) // GW

    with tile.TileContext(nc) as tc, ExitStack() as ctx:
        p_c = ctx.enter_context(tc.tile_pool(name="p_c", bufs=1))
        p_msg = ctx.enter_context(tc.tile_pool(name="p_msg", bufs=3))
        p_meta = ctx.enter_context(tc.tile_pool(name="p_meta", bufs=3))
        p_sel = ctx.enter_context(tc.tile_pool(name="p_sel", bufs=3))
        p_agg = ctx.enter_context(tc.tile_pool(name="p_agg", bufs=4))
        ps_agg = ctx.enter_context(tc.tile_pool(name="ps_agg", bufs=4,
                                                space="PSUM"))
        ps_out = ctx.enter_context(tc.tile_pool(name="ps_out", bufs=2,
                                                space="PSUM"))

        wt = p_c.tile([2 * IN_SIZE, 2 * OUT_SIZE], F32)
        nc.sync.dma_start(wt[:], wst[:])
        bt = p_c.tile([N_REL, OUT_SIZE], F32)
        nc.sync.dma_start(bt[:], biasd[:])
        ot = p_c.tile([N_REL, W], F32)
        nc.sync.dma_start(ot[:], onesd[:])
        iot = p_c.tile([P, GW * W], BF16)
        nc.sync.dma_start(iot[:], iotad[:])
        outsb = p_c.tile([P, n_wcol * OUT_SIZE], F32)

        groups = {}

        def get_group(g):
            if g not in groups:
                g0, g1 = g * GW, min((g + 1) * GW, T)
                nb = g1 - g0
                mt = p_msg.tile([P, GW * IN_SIZE], BF16, tag="msg")
                nc.scalar.dma_start(mt[:, :nb * IN_SIZE],
                                    emsg[:, g0 * IN_SIZE:g1 * IN_SIZE])
                wl = p_meta.tile([P, GW], BF16, tag="wlc")
                nc.sync.dma_start(wl[:, :nb], ewlc[:, g0:g1])
                vl = p_meta.tile([P, GW], BF16, tag="val")
                nc.sync.dma_start(vl[:, :nb], eval_[:, g0:g1])
                st = p_sel.tile([P, GW * W], BF16, tag="sel")
                s3 = st[:, :nb * W].rearrange("p (v j) -> p v j", v=nb)
                i3 = iot[:, :nb * W].rearrange("p (v j) -> p v j", v=nb)
                bw = wl[:, :nb].unsqueeze(-1).broadcast_to([P, nb, W])
                bv = vl[:, :nb].unsqueeze(-1).broadcast_to([P, nb, W])
                nc.vector.tensor_tensor(s3, i3, bw,
                                        op=mybir.AluOpType.is_equal)
                nc.vector.tensor_tensor(s3, s3, bv, op=mybir.AluOpType.mult)
                groups[g] = (mt, st)
            return groups[g]

        def cell_ap(b):
            mt, st = get_group(b // GW)
            o = b % GW
            return (mt[:, o * IN_SIZE:(o + 1) * IN_SIZE],
                    st[:, o * W:(o + 1) * W])

        for w in range(n_win):
            aggs = []
            for half in range(2):
                ps = ps_agg.tile([P, W], F32)
                for rr in range(2):
                    c2 = w * N_REL + 2 * half + rr
                    b0, b1 = int(starts[c2]), int(starts[c2 + 1])
                    for k, b in enumerate(range(b0, b1)):
                        m_ap, s_ap = cell_ap(b)
                        nc.tensor.matmul(
                            out=ps[rr * IN_SIZE:(rr + 1) * IN_SIZE, :],
                            lhsT=m_ap, rhs=s_ap,
                            tile_position=(0, rr * IN_SIZE),
                            start=(k == 0), stop=(b == b1 - 1))
                agg = p_agg.tile([P, W], F32, tag="agg")
                nc.vector.tensor_copy(agg[:], ps[:])
                aggs.append(agg)
            po = ps_out.tile([W, OUT_SIZE], F32)
            nc.tensor.matmul(out=po[:], lhsT=aggs[0][:],
                             rhs=wt[:, :OUT_SIZE], start=True, stop=False)
            nc.tensor.matmul(out=po[:], lhsT=aggs[1][:],
                             rhs=wt[:, OUT_SIZE:], start=False, stop=False)
            nc.tensor.matmul(out=po[:], lhsT=ot[:], rhs=bt[:],
                             start=False, stop=True)
            nc.scalar.copy(
                outsb[(w % 4) * W:(w % 4 + 1) * W,
                      (w // 4) * OUT_SIZE:(w // 4 + 1) * OUT_SIZE],
                po[:])
        nc.sync.dma_start(out[:], outsb[:])

    nc.compile()
    _PROG_CACHE[key] = nc
    return nc


def _prepare(inp, src, dst, edge_val, weights, bias):
    inp = np.asarray(inp, dtype=np.float32)
    src = np.asarray(src)
    dst = np.asarray(dst)
    edge_val = np.asarray(edge_val, dtype=np.float32)
    weights = np.asarray(weights, dtype=np.float32)
    bias = np.asarray(bias, dtype=np.float32)

    n_win, starts, T, msg_all, wloc_all, val_all = _host_prep(
        inp, src, dst, edge_val)
    nc = _build_program(n_win, starts, T)

    wst = np.zeros((2 * IN_SIZE, 2 * OUT_SIZE), dtype=np.float32)
    wst[:IN_SIZE, :OUT_SIZE] = weights[0]
    wst[IN_SIZE:, :OUT_SIZE] = weights[1]
    wst[:IN_SIZE, OUT_SIZE:] = weights[2]
    wst[IN_SIZE:, OUT_SIZE:] = weights[3]
    ones = np.ones((N_REL, W), dtype=np.float32)
    bf16 = _bf16()
    iota = np.broadcast_to(
        np.tile(np.arange(W, dtype=np.float32), GW).astype(bf16),
        (P, GW * W)).copy()

    in_maps = []
    for c in range(N_CORES):
        in_maps.append({
            "wst": wst,
            "biasd": bias,
            "onesd": ones,
            "iotad": iota,
            "emsg": msg_all[c].reshape(P, T * IN_SIZE),
            "ewlc": wloc_all[c],
            "eval": val_all[c],
        })
    return nc, in_maps, n_win


def _finish(res, n_win):
    n_wcol = (n_win + 3) // 4
    parts = []
    for c in range(N_CORES):
        arr = res.results[c]["out"].reshape(4, W, n_wcol, OUT_SIZE)
        nodes = arr.transpose(2, 0, 1, 3).reshape(n_wcol * 4 * W, OUT_SIZE)
        parts.append(nodes[:NPC])
    return np.concatenate(parts, axis=0).astype(np.float32)


def kernel(inp, src, dst, edge_val, weights, bias):
    nc, in_maps, n_win = _prepare(inp, src, dst, edge_val, weights, bias)
    res = run_bass_kernel_spmd(nc, in_maps, list(range(N_CORES)))
    return _finish(res, n_win)
